# revision 5
# baseline (speedup 1.0000x reference)
"""Trainium2 distributed kernel for CrossRNN (grid of 2-layer ReLU RNNs +
row/col message passing + linear head), 8 NeuronCores SPMD.

Math (per grid cell): 2-layer Elman RNN (relu) over S=32 embedded tokens,
last hidden h of the top layer, then with u = h.w1, s = h.w2:
  out[b,r,c] = u - 2*s + sum_c' s[b,r,c'] + sum_r' s[b,r',c] + pred_b

Sharding: core k owns sample b=k//2, rows [32*(k%2), 32*(k%2)+32) => 2048
independent sequences/core. Row sums are local; column sums need one pairwise
AllReduce of a [64]-float vector between cores (2b, 2b+1).

Per-core device pipeline:
 - Embedding table is fed as bf16 [30000, 128] in HBM; x indices are
   host-pre-wrapped into dma_gather's int16 [16-partition-wrapped] layout.
 - Per timestep, 4x gpsimd.dma_gather(transpose=True, 512 idxs) pull that
   step's embedding rows straight into [E=128 partitions, 2048] bf16 layout.
   The four calls rotate over 4 SWDGE queues (num_swdge_queues=4) so all four
   Q7 core-pairs generate descriptors concurrently - this is the kernel's
   throughput limiter (SWDGE desc-gen on GpSimd: 994ns fixed + 0.34ns/idx
   per call) and 4 queues cut the gather stream from ~610us to ~163us.
   A 3-call split (768,768,512) saves one fixed overhead per step (~8us
   total, 212us vs 220us). 896-idx calls CRASH the device (ring 58x2=116
   entries races the 128-deep SWDGE FIFO); 768 (54x2=108) is verified safe
   over multiple runs. ap_gather is NOT an alternative: ~56-65us per call
   regardless of size (scalar per-index Q7 processing; the trace slice
   shows only a ~0.4-4us tail, the rest hides as engine GAP). NOTE: >=3 queues exhibits a benign
   hardware-level nondeterminism (~1e-3-scale output jitter, rel err stays
   ~4.4e-3 vs the 2e-2 gate over many runs); 2 queues is bit-deterministic
   but ~160us slower. Gather calls >896 indices crash the device (SWDGE
   descriptor-ring limit) - keep 512.
 - TensorE per step: psum1[c] = W_ih0 @ g[c] (+ W_hh0 @ h1_prev[c]);
   relu+bias on ScalarE -> h1 (bf16); same for layer 2 with relu on VectorE;
   4 chunks of 512 columns pipeline PE against ACT/DVE. Weights are bf16
   lhsT (host-transposed); biases b_ih+b_hh are added on device and fused
   into the relu ops. PSUM: 4+4 banks double-buffer the two layers.
 - Head: pw=[w1 w2] matmul -> u,s rows in PSUM; s spread to [32 rows, 64
   cols]; col-sum partial via a ones-vector matmul; pairwise AllReduce
   (preceded by an early warmup AllReduce that hides the ~11us ncfw
   first-use trigger latency); row sums + final combine on VectorE overlap
   the collective.
"""

import numpy as np
import ml_dtypes

B, R, C, S = 4, 64, 64, 32
V, E, H, L = 30000, 128, 128, 2
N_CORES = 8
NPC = (B * R * C) // N_CORES  # 2048 sequences per core
ROWS_PC = 32                  # rows per core
NCH, CW = 4, 512              # column chunks for pipelining

_cache = {}

# tunables (bisection / perf knobs)
GATHER_SPLIT = (768, 768, 512)  # per-step dma_gather call sizes (sum = NPC)
N_STEPS = S           # timesteps actually executed (S for correct output)
USE_COLLECTIVE = True
NQ = 4


def _build():
    """Build + compile the Bass graph once per (pred_b is passed at runtime
    via the biases tensor, so the graph itself is input-independent)."""
    if "nc" in _cache:
        return _cache["nc"]

    import concourse.mybir as mybir
    import concourse.tile as tile
    from concourse import bacc
    from concourse.bass import ds

    f32 = mybir.dt.float32
    bf16 = mybir.dt.bfloat16
    i16 = mybir.dt.int16

    nc = bacc.Bacc("TRN2", target_bir_lowering=False, debug=False,
                   num_devices=N_CORES, num_swdge_queues=NQ)

    embed_d = nc.dram_tensor("embed", [V, E], bf16, kind="ExternalInput")
    idx_d = nc.dram_tensor("idx", [128, S * (NPC // 16)], i16, kind="ExternalInput")
    wts_d = nc.dram_tensor("wts", [128, 4 * H], bf16, kind="ExternalInput")
    # biases: cols 0..3 = b_ih0, b_hh0, b_ih1, b_hh1 ; col 4 = pred_b bcast
    biases_d = nc.dram_tensor("biases", [128, 5], f32, kind="ExternalInput")
    pw_d = nc.dram_tensor("pw", [128, 2], bf16, kind="ExternalInput")
    out_d = nc.dram_tensor("out", [ROWS_PC, C], f32, kind="ExternalOutput")

    with tile.TileContext(nc) as tc:
        with (
            tc.tile_pool(name="const", bufs=1) as constp,
            tc.tile_pool(name="gpool", bufs=5) as gpool,
            tc.tile_pool(name="h1p", bufs=2) as h1p,
            tc.tile_pool(name="h2p", bufs=2) as h2p,
            tc.tile_pool(name="tailp", bufs=1) as tailp,
            tc.tile_pool(name="dram", bufs=1, space="DRAM") as dramp,
        ):
            idx_sb = constp.tile([128, S * (NPC // 16)], i16)
            wts_sb = constp.tile([128, 4, H], bf16)
            biases_sb = constp.tile([128, 5], f32)
            pw_sb = constp.tile([128, 2], bf16)
            bias0 = constp.tile([128, 1], f32)
            bias1 = constp.tile([128, 1], f32)

            nc.sync.dma_start(idx_sb[:], idx_d.ap())
            nc.sync.dma_start(wts_sb[:, :, :], wts_d.ap().rearrange("k (w m) -> k w m", w=4))
            nc.sync.dma_start(biases_sb[:], biases_d.ap())
            nc.sync.dma_start(pw_sb[:], pw_d.ap())
            nc.vector.tensor_add(bias0[:], biases_sb[:, 0:1], biases_sb[:, 1:2])
            nc.vector.tensor_add(bias1[:], biases_sb[:, 2:3], biases_sb[:, 3:4])


            h1_prev = None
            h2_prev = None
            with (
                tc.tile_pool(name="p1p", bufs=4, space="PSUM") as p1p,
                tc.tile_pool(name="p2p", bufs=4, space="PSUM") as p2p,
            ):
                gq = 0
                for t in range(N_STEPS):
                    g = gpool.tile([128, 1, NPC], bf16, tag="g")
                    off = 0
                    for gc in GATHER_SPLIT:
                        nc.gpsimd.dma_gather(
                            g[:, :, ds(off, gc)], embed_d.ap(),
                            idx_sb[:, ds(t * (NPC // 16) + off // 16, gc // 16)],
                            gc, gc, E, transpose=True,
                            queue_num=gq % NQ,
                        )
                        off += gc
                        gq += 1
                    if t == 1:
                        # warmup collective emitted after step-0 gathers so it
                        # hides behind the gather stream instead of delaying it;
                        # it wakes ncfw so the tail AllReduce triggers fast
                        warm_in = dramp.tile([1, C], f32)
                        warm_out = dramp.tile([1, C], f32)
                        warm_sb = constp.tile([1, C], f32)
                        nc.vector.memset(warm_sb[:], 0.0)
                        nc.gpsimd.dma_start(warm_in[:], warm_sb[:])
                        nc.gpsimd.collective_compute(
                            "AllReduce", mybir.AluOpType.add,
                            replica_groups=[[0, 1], [2, 3], [4, 5], [6, 7]],
                            ins=[warm_in.opt()], outs=[warm_out.opt()],
                        )
                    h1_cur = h1p.tile([128, NPC], bf16, tag="h1")
                    h2_cur = h2p.tile([128, NPC], bf16, tag="h2")

                    p1s = []
                    for c in range(NCH):
                        p1 = p1p.tile([128, CW], f32, tag="p1")
                        nc.tensor.matmul(p1[:], wts_sb[:, 0, :], g[:, 0, ds(c * CW, CW)],
                                         start=True, stop=(t == 0))
                        if t > 0:
                            nc.tensor.matmul(p1[:], wts_sb[:, 1, :],
                                             h1_prev[:, ds(c * CW, CW)],
                                             start=False, stop=True)
                        nc.scalar.activation(h1_cur[:, ds(c * CW, CW)], p1[:],
                                             mybir.ActivationFunctionType.Relu,
                                             bias=bias0[:])
                        p1s.append(p1)

                    for c in range(NCH):
                        p2 = p2p.tile([128, CW], f32, tag="p2")
                        nc.tensor.matmul(p2[:], wts_sb[:, 2, :],
                                         h1_cur[:, ds(c * CW, CW)],
                                         start=True, stop=(t == 0))
                        if t > 0:
                            nc.tensor.matmul(p2[:], wts_sb[:, 3, :],
                                             h2_prev[:, ds(c * CW, CW)],
                                             start=False, stop=True)
                        nc.vector.tensor_scalar(h2_cur[:, ds(c * CW, CW)], p2[:],
                                                bias1[:], 0.0,
                                                mybir.AluOpType.add,
                                                mybir.AluOpType.max)
                    h1_prev, h2_prev = h1_cur, h2_cur

            # ---- head: u = h.w1, s = h.w2 (psum [2, NPC] in 512-chunks) ----
            us_sb = tailp.tile([2, NPC], f32)
            with tc.tile_pool(name="usp", bufs=2, space="PSUM") as usp:
                for c in range(NCH):
                    pus = usp.tile([2, CW], f32, tag="us")
                    nc.tensor.matmul(pus[:], pw_sb[:], h2_prev[:, ds(c * CW, CW)],
                                     start=True, stop=True)
                    nc.vector.tensor_copy(us_sb[:, ds(c * CW, CW)], pus[:])

            # spread s to [rows, cols]; col-sum via ones-matmul (fast), then
            # ship the partial to the pair core
            s_rc = tailp.tile([ROWS_PC, C], f32)
            nc.sync.dma_start(s_rc[:], us_sb[1:2, :].rearrange("p (r c) -> p r c", r=ROWS_PC))
            ones_sb = tailp.tile([ROWS_PC, 1], f32)
            nc.vector.memset(ones_sb[:], 1.0)
            colS_p = tailp.tile([1, C], f32)
            with tc.tile_pool(name="cspp", bufs=1, space="PSUM") as cspp:
                csp_ps = cspp.tile([1, C], f32)
                nc.tensor.matmul(csp_ps[:], ones_sb[:], s_rc[:], start=True, stop=True)
                nc.vector.tensor_copy(colS_p[:], csp_ps[:])
            cs_in = dramp.tile([1, C], f32)
            cs_out = dramp.tile([1, C], f32)
            nc.gpsimd.dma_start(cs_in[:], colS_p[:])
            if USE_COLLECTIVE:
                nc.gpsimd.collective_compute(
                    "AllReduce", mybir.AluOpType.add,
                    replica_groups=[[0, 1], [2, 3], [4, 5], [6, 7]],
                    ins=[cs_in.opt()], outs=[cs_out.opt()],
                )
            else:
                cs_out = cs_in
            colS_tot = tailp.tile([1, C], f32)
            nc.gpsimd.dma_start(colS_tot[:], cs_out[:])
            colS_bc = tailp.tile([ROWS_PC, C], f32)
            nc.gpsimd.partition_broadcast(colS_bc[:], colS_tot[:])

            # overlapped with the AllReduce: u spread, row sums (+pred_b), -2s+u
            u_rc = tailp.tile([ROWS_PC, C], f32)
            nc.sync.dma_start(u_rc[:], us_sb[0:1, :].rearrange("p (r c) -> p r c", r=ROWS_PC))
            rowS = tailp.tile([ROWS_PC, 1], f32)
            nc.vector.tensor_reduce(rowS[:], s_rc[:], axis=mybir.AxisListType.X,
                                    op=mybir.AluOpType.add)
            nc.vector.tensor_add(rowS[:], rowS[:], biases_sb[0:ROWS_PC, 4:5])
            acc = tailp.tile([ROWS_PC, C], f32)
            nc.vector.scalar_tensor_tensor(acc[:], s_rc[:], -2.0, u_rc[:],
                                           mybir.AluOpType.mult, mybir.AluOpType.add)
            nc.vector.tensor_scalar(acc[:], acc[:], rowS[:], None, mybir.AluOpType.add)
            nc.vector.tensor_tensor(acc[:], acc[:], colS_bc[:], mybir.AluOpType.add)
            nc.sync.dma_start(out_d.ap(), acc[:])

    nc.compile()
    _cache["nc"] = nc
    return nc


def _prep_in_maps(inputs):
    x = np.asarray(inputs["x"])
    embed = np.asarray(inputs["embed"], dtype=np.float32)
    W_ih = np.asarray(inputs["W_ih"], dtype=np.float32)
    W_hh = np.asarray(inputs["W_hh"], dtype=np.float32)
    b_ih = np.asarray(inputs["b_ih"], dtype=np.float32)
    b_hh = np.asarray(inputs["b_hh"], dtype=np.float32)
    pred_W = np.asarray(inputs["pred_W"], dtype=np.float32)
    pred_b = np.asarray(inputs["pred_b"], dtype=np.float32)

    embed_bf = np.ascontiguousarray(embed.astype(ml_dtypes.bfloat16))
    # lhsT layouts: [K(part) = input dim, M(free) = output dim] = W.T
    wts = np.stack([W_ih[0].T, W_hh[0].T, W_ih[1].T, W_hh[1].T], axis=1)  # [128,4,128]
    wts = np.ascontiguousarray(wts.reshape(128, 4 * H).astype(ml_dtypes.bfloat16))
    biases = np.stack(
        [b_ih[0], b_hh[0], b_ih[1], b_hh[1], np.full(H, pred_b[0], np.float32)],
        axis=1,
    ).astype(np.float32)  # [128, 5]
    pw = np.ascontiguousarray(pred_W[0].reshape(2, H).T.astype(ml_dtypes.bfloat16))

    in_maps = []
    for k in range(N_CORES):
        b, r0 = k // 2, ROWS_PC * (k % 2)
        xs = np.asarray(x[b, r0:r0 + ROWS_PC]).reshape(NPC, S).astype(np.int16)
        idx = np.empty((128, S * (NPC // 16)), np.int16)
        for t in range(S):
            wrapped = xs[:, t].reshape(NPC // 16, 16).T  # [16, NPC//16]
            idx[:, t * (NPC // 16):(t + 1) * (NPC // 16)] = np.tile(wrapped, (8, 1))
        in_maps.append({
            "embed": embed_bf, "idx": np.ascontiguousarray(idx),
            "wts": wts, "biases": biases, "pw": pw,
        })
    return in_maps


def run(inputs, trace=False):
    from concourse import bass_utils
    nc = _build()
    in_maps = _prep_in_maps(inputs)
    res = bass_utils.run_bass_kernel_spmd(
        nc, in_maps, core_ids=list(range(N_CORES)), trace=trace,
    )
    out = np.empty((B, R, C), np.float32)
    for k in range(N_CORES):
        b, r0 = k // 2, ROWS_PC * (k % 2)
        out[b, r0:r0 + ROWS_PC, :] = res.results[k]["out"]
    return out, res


def kernel(**inputs):
    out, _ = run(inputs, trace=False)
    return out

